# revision 10
# baseline (speedup 1.0000x reference)
"""MinGRU Trainium2 kernel (nn_MinGRUTriton_77309411812).

Reference computation (B=4, L=8192, D=1024, fp32):
    gates      = sigmoid(x @ Wg.T + bg)
    candidates = tanh   (x @ Wc.T + bc)
    h_t = gates_t * h_{t-1} + candidates_t        (h_0 = 0, scan along L)

Sharding (8 cores, no cross-core communication):
    core c -> batch b = c // 2, output-channel half eh = c % 2 (512 channels).

Mixed-precision matmuls: the candidate path and the upper half (kg 4-7)
of the gate contraction run fp16 (PE at 1 cycle/row); the lower half
(kg 0-3) of the gate contraction runs fp8-e4m3 with perf_mode=DoubleRow
(2 contraction rows/cycle, ~1.44x over fp16 at FD=512), accumulating
into the same fp32 PSUM bank.  Error budget: full-fp8 gates measure
2.57e-2 max-rel on the exact reference inputs; quantization error scales
as sqrt(alpha) with the fp8 contraction fraction, so alpha=0.5 lands at
~1.8e-2 against the 2e-2 gate (candidates stay fp16: their errors feed
the scan directly and measure 5.7e-2 if quantized).  fp8 operands are
quantized host-side from the fp16-staged values with ml_dtypes
float8_e4m3 (IEEE-style, max 240 = TRN FP8_EXP4).

Host-side shard prep feeds each core transposed operands in DMA-native
layouts so the device kernel needs no transposes or casts:
    xh [p, ci, kg, t]    = x16[b, ci*TC + t, kg*128 + p]        fp16
    xdr[p, ci, q, j, t]  = q8(x16)[b, ci*TC + t, (2q+j)*128+p]  fp8
    wgh[p, eg, kh, e]    = Wg16[eh*512+eg*128+e, (4+kh)*128+p]  fp16
    wgdr[p, eg, q, j, e] = q8(Wg16)[.., (2q+j)*128 + p]         fp8
    wch[p, eg, kg, e]    = Wc16[eh*512+eg*128+e, kg*128 + p]    fp16
(k on partitions; DoubleRow pairs j are two consecutive 128-row blocks,
matching the [128, 2, M] stationary / [128, 2, N] moving AP convention.)

The matmul output lands as [e(partitions), t(free)], exactly the layout
tensor_tensor_scan needs; h is stored fp16 in hh[p, ci, eg, t] and
un-permuted/upcast on the host.

Startup: weights+bias ride the scalar HWDGE ring, x chunks the sync
ring, issued in exact consumption order (startup demand sits at the HBM
roofline); dummy matmuls on a zeroed tile keep the PE's HAM clock gate
warming (~6us activity + ~2.4us transition stall releases the 1.2->2.4
GHz throttle) without consuming HBM bandwidth.  Mid-kernel h stores ride
the scalar ring (idle after weights); the last chunk's per-group stores
ride the then-idle sync ring so the kernel-tail store fires immediately.
"""

import sys

import numpy as np

try:
    import concourse.bass as bass  # noqa: F401
except ImportError:  # pragma: no cover - path fallback for fresh environments
    sys.path.insert(0, "/opt/trn_rl_repo")

import ml_dtypes

import concourse.bass as bass
import concourse.mybir as mybir
import concourse.tile as tile
from concourse import bacc
from concourse.bass_utils import run_bass_kernel_spmd
from concourse.tile import add_dep_helper

B, L, D = 4, 8192, 1024
E = D // 2          # output channels per core
N_CORES = 8
TC = 512            # t-chunk (= matmul moving free dim = PSUM bank)
NK = D // 128       # contraction k-groups
NKH = NK // 2       # fp16 gate k-groups (kg 4-7)
NQ = 2              # fp8 DoubleRow pair-groups (kg 0-3 as 2 x 256)
NE = E // 128       # output-channel groups per core
NCH = L // TC       # t-chunks

F32 = mybir.dt.float32
F16 = mybir.dt.float16
F8 = mybir.dt.float8e4
DR = mybir.MatmulPerfMode.DoubleRow

_compiled = None


def _build():
    nc = bacc.Bacc("TRN2", target_bir_lowering=False, debug=False)

    xh = nc.dram_tensor("xh", [128, NCH, NK, TC], F16, kind="ExternalInput")
    xdr = nc.dram_tensor("xdr", [128, NCH, NQ, 2, TC], F8, kind="ExternalInput")
    wgh = nc.dram_tensor("wgh", [128, NE, NKH, 128], F16, kind="ExternalInput")
    wgdr = nc.dram_tensor("wgdr", [128, NE, NQ, 2, 128], F8,
                          kind="ExternalInput")
    wch = nc.dram_tensor("wch", [128, NE, NK, 128], F16, kind="ExternalInput")
    bias = nc.dram_tensor("bias", [128, 2 * NE], F32, kind="ExternalInput")
    hh = nc.dram_tensor("hh", [128, NCH, NE, TC], F16, kind="ExternalOutput")

    with tile.TileContext(nc) as tc, \
            tc.tile_pool(name="wpool", bufs=1) as wpool, \
            tc.tile_pool(name="xpool", bufs=3) as xpool, \
            tc.tile_pool(name="gcpool", bufs=2) as gcpool, \
            tc.tile_pool(name="hpool", bufs=2) as hpool, \
            tc.tile_pool(name="pspool", bufs=7, space="PSUM") as pspool:

        b_all = wpool.tile([128, 2 * NE], F32)
        nc.scalar.dma_start(out=b_all[:], in_=bias[:])
        bg_t = b_all[:, 0:NE]
        bc_t = b_all[:, NE:2 * NE]
        # Weight issue order = consumption order: fp8 gate weights (first
        # matmuls of chunk 0), fp16 gate weights per-eg, candidate
        # weights per-eg.  Each dma_start costs ~600ns of descriptor
        # generation (DIRECT2D) on its queue, so pieces are kept coarse.
        wgdr_t = wpool.tile([128, NE, NQ, 2, 128], F8)
        nc.scalar.dma_start(out=wgdr_t[:], in_=wgdr[:])
        wg_t = wpool.tile([128, NE, NKH, 128], F16)
        i_wg_pieces = []
        for eg in range(NE):
            i_wg = nc.scalar.dma_start(out=wg_t[:, eg], in_=wgh[:, eg])
            i_wg_pieces.append(i_wg)
        wc_t = wpool.tile([128, NE, NK, 128], F16)
        i_wc_pieces = []
        for eg in range(NE):
            i_wc = nc.scalar.dma_start(out=wc_t[:, eg], in_=wch[:, eg])
            i_wc_pieces.append(i_wc)

        # Warm the PE's HAM clock gate with dummy matmuls on a zeroed
        # tile while the startup DMAs are in flight; dummies consume no
        # HBM bandwidth, which the startup transfers need entirely.
        warm = wpool.tile([128, 512], F16)
        nc.vector.memset(warm[:], 0.0)
        warm_ps = pspool.tile([128, 512], F32, tag="warm", bufs=1)
        for _ in range(12):
            nc.tensor.matmul(warm_ps[:], warm[:, 0:128], warm[:, 0:512],
                             start=True, stop=True)

        h_prev = None
        for ci in range(NCH):
            xd_t = xpool.tile([128, NQ, 2, TC], F8, tag="xd")
            i_xd = nc.sync.dma_start(out=xd_t[:], in_=xdr[:, ci])
            if ci == 1:
                add_dep_helper(i_xd.ins, i_wg_pieces[2].ins,
                               reason="defer xd1 behind wg eg2")
            x_t = xpool.tile([128, NK, TC], F16, tag="x")
            if ci == 1:
                i_x = nc.sync.dma_start(out=x_t[:, 0:NK // 2],
                                        in_=xh[:, ci, 0:NK // 2])
                nc.sync.dma_start(out=x_t[:, NK // 2:],
                                  in_=xh[:, ci, NK // 2:])
            else:
                i_x = nc.sync.dma_start(out=x_t[:], in_=xh[:, ci])
                if ci == 2:
                    add_dep_helper(i_x.ins, i_wc_pieces[1].ins,
                                   reason="defer x2 behind wc eg1")

            g_t = gcpool.tile([128, NE, TC], F32, tag="g")
            c_t = gcpool.tile([128, NE, TC], F32, tag="c")
            h_t = hpool.tile([128, NE, TC], F16, tag="h")
            last = ci == NCH - 1

            SIG = mybir.ActivationFunctionType.Sigmoid
            TANH = mybir.ActivationFunctionType.Tanh

            def gate_unit(eg):
                ps = pspool.tile([128, TC], F32, tag="ps", name="ps")
                for q in range(NQ):
                    nc.tensor.matmul(
                        ps[:],
                        wgdr_t[:, eg, q],
                        xd_t[:, q],
                        start=(q == 0),
                        stop=False,
                        perf_mode=DR,
                    )
                for kh in range(NKH):
                    nc.tensor.matmul(
                        ps[:],
                        wg_t[:, eg, kh],
                        x_t[:, NKH + kh],
                        start=False,
                        stop=(kh == NKH - 1),
                    )
                nc.scalar.activation(
                    g_t[:, eg], ps[:], SIG, bias=bg_t[:, eg:eg + 1],
                )

            def cand_unit(eg, pieces):
                # One PSUM tile per piece: sharing a tile would serialize
                # the second piece's matmuls behind the first piece's
                # activation read of the same PSUM bank.
                for toff, tcw in pieces:
                    ps = pspool.tile([128, TC], F32, tag="ps", name="ps")
                    for kg in range(NK):
                        nc.tensor.matmul(
                            ps[:, 0:tcw],
                            wc_t[:, eg, kg],
                            x_t[:, kg, toff:toff + tcw],
                            start=(kg == 0),
                            stop=(kg == NK - 1),
                        )
                    nc.scalar.activation(
                        c_t[:, eg, toff:toff + tcw], ps[:, 0:tcw],
                        TANH, bias=bc_t[:, eg:eg + 1],
                    )

            whole = ((0, TC),)
            # The very last unit + scan of the kernel run as two 256-wide
            # halves so the final MM->ACT->scan->store chain is half as
            # long.
            halved = ((0, TC // 2), (TC // 2, TC // 2))
            for eg in range(NE):
                gate_unit(eg)
            for eg in range(NE):
                cand_unit(eg, halved if last and eg == NE - 1 else whole)

            for eg in range(NE):
                pieces = halved if last and eg == NE - 1 else whole
                for toff, tcw in pieces:
                    if toff == 0:
                        init = 0.0 if ci == 0 else h_prev[:, eg, TC - 1:TC]
                    else:
                        init = h_t[:, eg, toff - 1:toff]
                    nc.vector.tensor_tensor_scan(
                        h_t[:, eg, toff:toff + tcw],
                        g_t[:, eg, toff:toff + tcw],
                        c_t[:, eg, toff:toff + tcw],
                        initial=init,
                        op0=mybir.AluOpType.mult,
                        op1=mybir.AluOpType.add,
                    )
                    if last:
                        # Per-group stores on the sync ring (idle at the
                        # tail: x loads are done issuing ~20us earlier),
                        # so the final store's descriptor generation
                        # neither waits behind scalar-queue activations
                        # nor blocks them.
                        nc.sync.dma_start(
                            out=hh[:, ci, eg, toff:toff + tcw],
                            in_=h_t[:, eg, toff:toff + tcw],
                        )
            if not last:
                # Mid-kernel stores ride the scalar ring (idle once the
                # weights are in): sharing the sync ring with the x loads
                # couples the store doorbells to the x-load FIFO and
                # fires them ~2 chunks late.
                nc.scalar.dma_start(out=hh[:, ci], in_=h_t[:])
            h_prev = h_t

    nc.compile()
    return nc


def _get_compiled():
    global _compiled
    if _compiled is None:
        _compiled = _build()
    return _compiled


def _q8(a):
    """fp32 -> TRN e4m3 (max 240) -> fp8 bytes via ml_dtypes float8_e4m3."""
    return np.clip(a, -240.0, 240.0).astype(ml_dtypes.float8_e4m3)


def make_in_maps(x, Wg, bg, Wc, bc):
    x16 = np.asarray(x, dtype=np.float32).astype(np.float16)
    # xh[p, ci, kg, t] = x16[b, ci*TC + t, kg*128 + p]
    xhs = []
    xdrs = []
    for b in range(B):
        xb = x16[b].reshape(NCH, TC, NK, 128)
        xhs.append(np.ascontiguousarray(xb.transpose(3, 0, 2, 1)))
        # xdr[p, ci, q, j, t] = q8(x16)[b, ci*TC + t, (2q+j)*128 + p]
        xq = _q8(xb[:, :, 0:2 * NQ, :].astype(np.float32))
        xdrs.append(np.ascontiguousarray(
            xq.reshape(NCH, TC, NQ, 2, 128).transpose(4, 0, 2, 3, 1)))
    in_maps = []
    for c in range(N_CORES):
        b, eh = divmod(c, 2)
        sl = slice(eh * E, (eh + 1) * E)
        wg16 = np.asarray(Wg, np.float32)[sl].astype(np.float16)
        wc16 = np.asarray(Wc, np.float32)[sl].astype(np.float16)
        # wgh[p, eg, kh, e'] = Wg16[eg*128 + e', (NKH+kh)*128 + p]
        wgh = np.ascontiguousarray(
            wg16[:, NKH * 128:].reshape(NE, 128, NKH, 128)
            .transpose(3, 0, 2, 1))
        # wgdr[p, eg, q, j, e'] = q8(Wg16)[eg*128 + e', (2q+j)*128 + p]
        wgdr = np.ascontiguousarray(
            _q8(wg16[:, 0:NKH * 128].astype(np.float32))
            .reshape(NE, 128, NQ, 2, 128).transpose(4, 0, 2, 3, 1))
        wch = np.ascontiguousarray(
            wc16.reshape(NE, 128, NK, 128).transpose(3, 0, 2, 1))
        in_maps.append({
            "xh": xhs[b],
            "xdr": xdrs[b],
            "wgh": wgh,
            "wgdr": wgdr,
            "wch": wch,
            "bias": np.ascontiguousarray(np.stack(
                [np.asarray(bg, np.float32)[sl].reshape(NE, 128),
                 np.asarray(bc, np.float32)[sl].reshape(NE, 128)],
            ).reshape(2 * NE, 128).T),
        })
    return in_maps


def assemble_output(results):
    out = np.empty((B, L, D), np.float32)
    for c in range(N_CORES):
        b, eh = divmod(c, 2)
        hhv = results[c]["hh"]  # [128, NCH, NE, TC] fp16
        # out[b, ci*TC + t, eh*E + eg*128 + p] = hh[p, ci, eg, t]
        out[b, :, eh * E:(eh + 1) * E] = (
            hhv.transpose(1, 3, 2, 0).reshape(L, E).astype(np.float32))
    return out


def kernel(x, Wg, bg, Wc, bc, _trace=False, _trace_kwargs=None):
    nc = _get_compiled()
    in_maps = make_in_maps(x, Wg, bg, Wc, bc)
    res = run_bass_kernel_spmd(
        nc, in_maps, list(range(N_CORES)), trace=_trace,
        **(_trace_kwargs or {}),
    )
    out = assemble_output(res.results)
    if _trace:
        kernel.last_results = res
    return out


# revision 12
# speedup vs baseline: 1.0074x; 1.0074x over previous
"""MinGRU Trainium2 kernel (nn_MinGRUTriton_77309411812).

Reference computation (B=4, L=8192, D=1024, fp32):
    gates      = sigmoid(x @ Wg.T + bg)
    candidates = tanh   (x @ Wc.T + bc)
    h_t = gates_t * h_{t-1} + candidates_t        (h_0 = 0, scan along L)

Sharding (8 cores, no cross-core communication):
    core c -> batch b = c // 2, output-channel half eh = c % 2 (512 channels).

Mixed-precision matmuls: the candidate path and the upper half (kg 4-7)
of the gate contraction run fp16 (PE at 1 cycle/row); the lower half
(kg 0-3) of the gate contraction runs fp8-e4m3 with perf_mode=DoubleRow
(2 contraction rows/cycle, ~1.44x over fp16 at FD=512), accumulating
into the same fp32 PSUM bank.  Error budget: full-fp8 gates measure
2.57e-2 max-rel on the exact reference inputs; quantization error scales
as sqrt(alpha) with the fp8 contraction fraction, so alpha=0.5 lands at
~1.8e-2 against the 2e-2 gate (candidates stay fp16: their errors feed
the scan directly and measure 5.7e-2 if quantized).  fp8 operands are
quantized host-side from the fp16-staged values with ml_dtypes
float8_e4m3 (IEEE-style, max 240 = TRN FP8_EXP4).

Host-side shard prep feeds each core transposed operands in DMA-native
layouts so the device kernel needs no transposes or casts:
    xh [p, ci, kg, t]    = x16[b, ci*TC + t, kg*128 + p]        fp16
    xdr[p, ci, q, j, t]  = q8(x16)[b, ci*TC + t, (2q+j)*128+p]  fp8
    wgh[p, eg, kh, e]    = Wg16[eh*512+eg*128+e, (4+kh)*128+p]  fp16
    wgdr[p, eg, q, j, e] = q8(Wg16)[.., (2q+j)*128 + p]         fp8
    wch[p, eg, kg, e]    = Wc16[eh*512+eg*128+e, kg*128 + p]    fp16
(k on partitions; DoubleRow pairs j are two consecutive 128-row blocks,
matching the [128, 2, M] stationary / [128, 2, N] moving AP convention.)

The matmul output lands as [e(partitions), t(free)], exactly the layout
tensor_tensor_scan needs; h is stored fp16 in hh[p, ci, eg, t] and
un-permuted/upcast on the host.

Startup: weights+bias ride the scalar HWDGE ring, x chunks the sync
ring, issued in exact consumption order (startup demand sits at the HBM
roofline); dummy matmuls on a zeroed tile keep the PE's HAM clock gate
warming (~6us activity + ~2.4us transition stall releases the 1.2->2.4
GHz throttle) without consuming HBM bandwidth.  Mid-kernel h stores ride
the scalar ring (idle after weights); the last chunk's per-group stores
ride the then-idle sync ring so the kernel-tail store fires immediately.
"""

import sys

import numpy as np

try:
    import concourse.bass as bass  # noqa: F401
except ImportError:  # pragma: no cover - path fallback for fresh environments
    sys.path.insert(0, "/opt/trn_rl_repo")

import ml_dtypes

import concourse.bass as bass
import concourse.mybir as mybir
import concourse.tile as tile
from concourse import bacc
from concourse.bass_utils import run_bass_kernel_spmd
from concourse.tile import add_dep_helper

B, L, D = 4, 8192, 1024
E = D // 2          # output channels per core
N_CORES = 8
TC = 512            # t-chunk (= matmul moving free dim = PSUM bank)
NK = D // 128       # contraction k-groups
NKH = NK // 2       # fp16 gate k-groups (kg 4-7)
NQ = 2              # fp8 DoubleRow pair-groups (kg 0-3 as 2 x 256)
NE = E // 128       # output-channel groups per core
NCH = L // TC       # t-chunks

F32 = mybir.dt.float32
F16 = mybir.dt.float16
F8 = mybir.dt.float8e4
DR = mybir.MatmulPerfMode.DoubleRow

_compiled = None


def _build():
    nc = bacc.Bacc("TRN2", target_bir_lowering=False, debug=False)

    xh = nc.dram_tensor("xh", [128, NCH, NK, TC], F16, kind="ExternalInput")
    xdr = nc.dram_tensor("xdr", [128, NCH, NQ, 2, TC], F8, kind="ExternalInput")
    wgh = nc.dram_tensor("wgh", [128, NE, NKH, 128], F16, kind="ExternalInput")
    wgdr = nc.dram_tensor("wgdr", [128, NE, NQ, 2, 128], F8,
                          kind="ExternalInput")
    wch = nc.dram_tensor("wch", [128, NE, NK, 128], F16, kind="ExternalInput")
    bias = nc.dram_tensor("bias", [128, 2 * NE], F32, kind="ExternalInput")
    hh = nc.dram_tensor("hh", [128, NCH, NE, TC], F16, kind="ExternalOutput")

    with tile.TileContext(nc) as tc, \
            tc.tile_pool(name="wpool", bufs=1) as wpool, \
            tc.tile_pool(name="xpool", bufs=3) as xpool, \
            tc.tile_pool(name="gcpool", bufs=2) as gcpool, \
            tc.tile_pool(name="hpool", bufs=2) as hpool, \
            tc.tile_pool(name="pspool", bufs=7, space="PSUM") as pspool:

        b_all = wpool.tile([128, 2 * NE], F32)
        nc.scalar.dma_start(out=b_all[:], in_=bias[:])
        bg_t = b_all[:, 0:NE]
        bc_t = b_all[:, NE:2 * NE]
        # Weight issue order = consumption order: fp8 gate weights (first
        # matmuls of chunk 0), fp16 gate weights per-eg, candidate
        # weights per-eg.  Each dma_start costs ~600ns of descriptor
        # generation (DIRECT2D) on its queue, so pieces are kept coarse.
        wgdr_t = wpool.tile([128, NE, NQ, 2, 128], F8)
        nc.scalar.dma_start(out=wgdr_t[:], in_=wgdr[:])
        wg_t = wpool.tile([128, NE, NKH, 128], F16)
        i_wg_pieces = []
        for eg in range(NE):
            i_wg = nc.scalar.dma_start(out=wg_t[:, eg], in_=wgh[:, eg])
            i_wg_pieces.append(i_wg)
        wc_t = wpool.tile([128, NE, NK, 128], F16)
        i_wc_pieces = []
        for eg in range(NE):
            i_wc = nc.scalar.dma_start(out=wc_t[:, eg], in_=wch[:, eg])
            i_wc_pieces.append(i_wc)

        # Warm the PE's HAM clock gate with dummy matmuls on a zeroed
        # tile while the startup DMAs are in flight; dummies consume no
        # HBM bandwidth, which the startup transfers need entirely.
        warm = wpool.tile([128, 512], F16)
        nc.vector.memset(warm[:], 0.0)
        warm_ps = pspool.tile([128, 512], F32, tag="warm", bufs=1)
        for _ in range(16):
            nc.tensor.matmul(warm_ps[:], warm[:, 0:128], warm[:, 0:512],
                             start=True, stop=True)

        h_prev = None
        for ci in range(NCH):
            xd_t = xpool.tile([128, NQ, 2, TC], F8, tag="xd")
            i_xd = nc.sync.dma_start(out=xd_t[:], in_=xdr[:, ci])
            if ci == 1:
                add_dep_helper(i_xd.ins, i_wg_pieces[2].ins,
                               reason="defer xd1 behind wg eg2")
            x_t = xpool.tile([128, NK, TC], F16, tag="x")
            if ci == 1:
                i_x = nc.sync.dma_start(out=x_t[:, 0:NK // 2],
                                        in_=xh[:, ci, 0:NK // 2])
                nc.sync.dma_start(out=x_t[:, NK // 2:],
                                  in_=xh[:, ci, NK // 2:])
            else:
                i_x = nc.sync.dma_start(out=x_t[:], in_=xh[:, ci])
                if ci == 2:
                    add_dep_helper(i_x.ins, i_wc_pieces[1].ins,
                                   reason="defer x2 behind wc eg1")

            # fp16 g/c: DVE runs 2x on 16-bit I/O (the scan recurrence is
            # fp32 internally); costs ~2.5e-3 relative on g, negligible
            # in quadrature against the fp8-gate term.
            g_t = gcpool.tile([128, NE, TC], F16, tag="g")
            c_t = gcpool.tile([128, NE, TC], F16, tag="c")
            h_t = hpool.tile([128, NE, TC], F16, tag="h")
            last = ci == NCH - 1

            SIG = mybir.ActivationFunctionType.Sigmoid
            TANH = mybir.ActivationFunctionType.Tanh

            def gate_unit(eg):
                ps = pspool.tile([128, TC], F32, tag="ps", name="ps")
                for q in range(NQ):
                    nc.tensor.matmul(
                        ps[:],
                        wgdr_t[:, eg, q],
                        xd_t[:, q],
                        start=(q == 0),
                        stop=False,
                        perf_mode=DR,
                    )
                for kh in range(NKH):
                    nc.tensor.matmul(
                        ps[:],
                        wg_t[:, eg, kh],
                        x_t[:, NKH + kh],
                        start=False,
                        stop=(kh == NKH - 1),
                    )
                nc.scalar.activation(
                    g_t[:, eg], ps[:], SIG, bias=bg_t[:, eg:eg + 1],
                )

            def cand_unit(eg, pieces):
                # One PSUM tile per piece: sharing a tile would serialize
                # the second piece's matmuls behind the first piece's
                # activation read of the same PSUM bank.
                for toff, tcw in pieces:
                    ps = pspool.tile([128, TC], F32, tag="ps", name="ps")
                    for kg in range(NK):
                        nc.tensor.matmul(
                            ps[:, 0:tcw],
                            wc_t[:, eg, kg],
                            x_t[:, kg, toff:toff + tcw],
                            start=(kg == 0),
                            stop=(kg == NK - 1),
                        )
                    nc.scalar.activation(
                        c_t[:, eg, toff:toff + tcw], ps[:, 0:tcw],
                        TANH, bias=bc_t[:, eg:eg + 1],
                    )

            whole = ((0, TC),)
            # The very last unit + scan of the kernel run as two 256-wide
            # halves so the final MM->ACT->scan->store chain is half as
            # long.
            halved = ((0, TC // 2), (TC // 2, TC // 2))
            for eg in range(NE):
                gate_unit(eg)
            for eg in range(NE):
                cand_unit(eg, halved if last and eg == NE - 1 else whole)

            for eg in range(NE):
                pieces = halved if last and eg == NE - 1 else whole
                for toff, tcw in pieces:
                    if toff == 0:
                        init = 0.0 if ci == 0 else h_prev[:, eg, TC - 1:TC]
                    else:
                        init = h_t[:, eg, toff - 1:toff]
                    nc.vector.tensor_tensor_scan(
                        h_t[:, eg, toff:toff + tcw],
                        g_t[:, eg, toff:toff + tcw],
                        c_t[:, eg, toff:toff + tcw],
                        initial=init,
                        op0=mybir.AluOpType.mult,
                        op1=mybir.AluOpType.add,
                    )
                    if last:
                        # Per-group stores on the sync ring (idle at the
                        # tail: x loads are done issuing ~20us earlier),
                        # so the final store's descriptor generation
                        # neither waits behind scalar-queue activations
                        # nor blocks them.
                        nc.sync.dma_start(
                            out=hh[:, ci, eg, toff:toff + tcw],
                            in_=h_t[:, eg, toff:toff + tcw],
                        )
            if not last:
                # Mid-kernel stores ride the scalar ring (idle once the
                # weights are in): sharing the sync ring with the x loads
                # couples the store doorbells to the x-load FIFO and
                # fires them ~2 chunks late.
                nc.scalar.dma_start(out=hh[:, ci], in_=h_t[:])
            h_prev = h_t

    nc.compile()
    return nc


def _get_compiled():
    global _compiled
    if _compiled is None:
        _compiled = _build()
    return _compiled


def _q8(a):
    """fp32 -> TRN e4m3 (max 240) -> fp8 bytes via ml_dtypes float8_e4m3."""
    return np.clip(a, -240.0, 240.0).astype(ml_dtypes.float8_e4m3)


def make_in_maps(x, Wg, bg, Wc, bc):
    x16 = np.asarray(x, dtype=np.float32).astype(np.float16)
    # xh[p, ci, kg, t] = x16[b, ci*TC + t, kg*128 + p]
    xhs = []
    xdrs = []
    for b in range(B):
        xb = x16[b].reshape(NCH, TC, NK, 128)
        xhs.append(np.ascontiguousarray(xb.transpose(3, 0, 2, 1)))
        # xdr[p, ci, q, j, t] = q8(x16)[b, ci*TC + t, (2q+j)*128 + p]
        xq = _q8(xb[:, :, 0:2 * NQ, :].astype(np.float32))
        xdrs.append(np.ascontiguousarray(
            xq.reshape(NCH, TC, NQ, 2, 128).transpose(4, 0, 2, 3, 1)))
    in_maps = []
    for c in range(N_CORES):
        b, eh = divmod(c, 2)
        sl = slice(eh * E, (eh + 1) * E)
        wg16 = np.asarray(Wg, np.float32)[sl].astype(np.float16)
        wc16 = np.asarray(Wc, np.float32)[sl].astype(np.float16)
        # wgh[p, eg, kh, e'] = Wg16[eg*128 + e', (NKH+kh)*128 + p]
        wgh = np.ascontiguousarray(
            wg16[:, NKH * 128:].reshape(NE, 128, NKH, 128)
            .transpose(3, 0, 2, 1))
        # wgdr[p, eg, q, j, e'] = q8(Wg16)[eg*128 + e', (2q+j)*128 + p]
        wgdr = np.ascontiguousarray(
            _q8(wg16[:, 0:NKH * 128].astype(np.float32))
            .reshape(NE, 128, NQ, 2, 128).transpose(4, 0, 2, 3, 1))
        wch = np.ascontiguousarray(
            wc16.reshape(NE, 128, NK, 128).transpose(3, 0, 2, 1))
        in_maps.append({
            "xh": xhs[b],
            "xdr": xdrs[b],
            "wgh": wgh,
            "wgdr": wgdr,
            "wch": wch,
            "bias": np.ascontiguousarray(np.stack(
                [np.asarray(bg, np.float32)[sl].reshape(NE, 128),
                 np.asarray(bc, np.float32)[sl].reshape(NE, 128)],
            ).reshape(2 * NE, 128).T),
        })
    return in_maps


def assemble_output(results):
    out = np.empty((B, L, D), np.float32)
    for c in range(N_CORES):
        b, eh = divmod(c, 2)
        hhv = results[c]["hh"]  # [128, NCH, NE, TC] fp16
        # out[b, ci*TC + t, eh*E + eg*128 + p] = hh[p, ci, eg, t]
        out[b, :, eh * E:(eh + 1) * E] = (
            hhv.transpose(1, 3, 2, 0).reshape(L, E).astype(np.float32))
    return out


def kernel(x, Wg, bg, Wc, bc, _trace=False, _trace_kwargs=None):
    nc = _get_compiled()
    in_maps = make_in_maps(x, Wg, bg, Wc, bc)
    res = run_bass_kernel_spmd(
        nc, in_maps, list(range(N_CORES)), trace=_trace,
        **(_trace_kwargs or {}),
    )
    out = assemble_output(res.results)
    if _trace:
        kernel.last_results = res
    return out
